# revision 1
# baseline (speedup 1.0000x reference)
"""Trainium2 Bass kernel for the non-local attention block (dense_transformer).

Reference computation per batch item b (x: [B=32, C=64, H=32, W=32], N=1024):
    xf    = x[b] reshaped [C, N]
    phi   = w_phi   @ xf                     [C, N]
    theta = (w_theta @ xf)^T                 [N, C]
    g     = (w_g @ xf)^T @ w_mv^T            [N, C]
    att   = theta @ phi                      [N, N]
    att   = att @ w_mk^T                     [N, N]
    att   = softmax(att, axis over rows n)
    out   = att @ g                          [N, C]
    final = w_mask @ out^T + xf              [C, N]

Key algebraic restructure: (theta @ phi) @ w_mk^T == theta @ (phi @ w_mk^T),
which removes the N^3 matmul (1073M MACs -> 2x67M MACs per batch).  The
softmax denominator divide is folded into the small g factor (64 wide)
instead of the [N, N] attention matrix.

Per-core layout (data-parallel, 4 batch items per core, processed as 2
stacked pairs occupying the 128 SBUF partitions; batch "b" on partitions
0-63, batch "c" on 64-127, PE quadrant tile-position packing runs both
batches' matmuls concurrently):
    T    = w_theta @ xf          [64, 1024]  (diag-quadrant pair matmuls)
    PhiT = xf^T @ w_phi^T        [1024, 64]  (row-split pair matmuls)
    GT   = xf^T @ (w_mv@w_g)^T   [1024, 64]  (row-split)
    P2   = PhiT^T @ w_mk^T       [64, 1024]  (col-split, accum over 8 m-chunks)
    S    = P2^T @ T              [1024, 1024] = att2^T  (row-split per k-chunk)
    E    = exp(S)  (ScalarE, fused row-sum via accum_out -> D)
    GTs  = GT * (1/D)            (fold softmax divide into g)
    O    = GTs^T @ E             [64, 1024]  (col-split, accum over m-chunks)
    final= w_mask @ O + xf       (diag-quadrant + DVE add)

All matmul operands bf16 (PE full rate); PSUM accumulation fp32; softmax
sum in fp32 via activation accum_out.  Weights are pre-transposed/cast on
host and replicated to all 8 cores.

PSUM budget (8 banks): S/exp pipeline 2 slots x [128,1024] = 4 banks;
P2 quarter-chunks + O accumulators share a 2-slot pool = 2 banks;
stage-1/mask psums rotate through another 2-slot pool = 2 banks.

Post-passes: _eliminate_redundant_waits strips Tile's transitively-implied
same-engine sem waits (they serialize the PE pipeline and block quadrant
concurrency); _split_matmul_waits hoists remaining multi-wait instructions
onto single-wait NoOps (TRN2 walrus allows one sync-wait per instruction).

Measured on TRN2 via axon NTFF profile: 87-90 us.  The PE HAM clock
gate can throttle whole phases depending on run alignment and device
state; the explicit O-after-next-S dependency edges (see o_chunk) keep
cold O matmuls from head-of-line-blocking the exp-feeding S chain in the
PE FIFO, which cut the throttled-mode time from ~100 to ~90 us.
Rel err ~1.0e-2 vs the fp32 reference.
"""

import numpy as np
import ml_dtypes

import concourse.bass as bass
import concourse.mybir as mybir
import concourse.tile as tile
from concourse.bass_utils import run_bass_kernel_spmd

BF = mybir.dt.bfloat16
F32 = mybir.dt.float32
EXP = mybir.ActivationFunctionType.Exp

B, C, HH, WW = 32, 64, 32, 32
N = HH * WW          # 1024
NCORES = 8
BPC = B // NCORES    # 4 batch items per core
NPAIRS = BPC // 2    # 2 stacked pairs per core
NK = N // 128        # 8 chunks of 128 along the N dimension
NH = 512             # matmul free-dim half (one PSUM bank)


def _build_body(nc, tc, consts, acts, bigacts, psO_pool, psS, psSm,
                xall32, xall16, wsmallT, wmkhT, out_e):
    lo = slice(0, 64)
    hi = slice(64, 128)

    # ---- PE warmup: dummy matmuls on an uninitialized tile keep the PE
    # busy for the HAM SHORT window (~3.4us) while the input DMAs run, so
    # real work starts at 2.4 GHz instead of 1.2.
    warm_in = consts.tile([128, 256], BF, tag="warm_in")
    nc.gpsimd.memset(warm_in[:], 0.0)
    warm_ps = psS.tile([128, N], F32, tag="psS", name="warm_ps")
    for i in range(40):
        nc.tensor.matmul(warm_ps[:, 0:128], lhsT=warm_in[:, 0:128],
                         rhs=warm_in[:, 128:256])

    # ---- inputs: few large DMAs, split across SP and GpSimd SWDGE rings
    # (each dma_start costs ~1us of sequencer issue time).
    wsmall = consts.tile([128, 4 * C], BF, tag="wsmall")
    nc.sync.dma_start(wsmall[:], wsmallT[:])
    wth = wsmall[:, 0 * C:1 * C]
    wph = wsmall[:, 1 * C:2 * C]
    wgv = wsmall[:, 2 * C:3 * C]
    wma = wsmall[:, 3 * C:4 * C]

    # All input DMAs on the SP ring in priority order: HW queues serve
    # descriptors FIFO per queue, so earlier-pushed transfers complete
    # first.  xball gates stage-1, wmk h0 gates P2, xfall only the final
    # residual add.  (A second ring would interleave descriptors and delay
    # the critical x transfer.)
    xball = acts.tile([128, NPAIRS, N], BF, tag="xball")
    nc.sync.dma_start(xball[:], xall16.rearrange("(p q) n -> q p n", p=NPAIRS))
    # wmk^T in k-quarter-major DRAM layout [4, 1024(m), 256(k)]: one DMA
    # piece per quarter so P2 quarter j (and then S chunk 2j's exp) starts
    # as soon as piece j lands instead of waiting for the full 2MB.
    wmk_q = []
    for j in range(4):
        t = consts.tile([128, NK, 256], BF, tag=f"wmkq{j}")
        nc.sync.dma_start(
            t[:], wmkhT[j * N:(j + 1) * N, :].rearrange(
                "(mc q) k -> q mc k", mc=NK))
        wmk_q.append(t)
    xfall = acts.tile([128, NPAIRS, N], F32, tag="xfall")
    nc.sync.dma_start(xfall[:], xall32.rearrange("(p q) n -> q p n", p=NPAIRS))

    st = [dict() for _ in range(NPAIRS)]

    def stage1(p):
        """PhiT/T/GT (quadrant-packed) for pair p."""
        xb = xball[:, p, :]
        s = st[p]
        psPhiT_b = psSm.tile([128, NH], F32, tag="psSm", name="psPhiT_b")
        psPhiT_c = psSm.tile([128, NH], F32, tag="psSm", name="psPhiT_c")
        for m in range(NK):
            mm = slice(m * 128, (m + 1) * 128)
            cc = slice(m * C, (m + 1) * C)
            nc.tensor.matmul(psPhiT_b[:, cc], lhsT=xb[lo, mm], rhs=wph[lo, :])
            nc.tensor.matmul(psPhiT_c[:, cc], lhsT=xb[hi, mm], rhs=wph[hi, :])
        PhiT_b = acts.tile([128, NH], BF, tag="PhiT_b", name="PhiT_b")
        PhiT_c = acts.tile([128, NH], BF, tag="PhiT_c", name="PhiT_c")
        nc.vector.tensor_copy(out=PhiT_b[:], in_=psPhiT_b[:])
        nc.vector.tensor_copy(out=PhiT_c[:], in_=psPhiT_c[:])

        T_sb = acts.tile([128, N], BF, tag="T_sb", name="T_sb")
        for h in range(2):
            hh = slice(h * NH, (h + 1) * NH)
            psT = psSm.tile([128, NH], F32, tag="psSm", name="psT")
            nc.tensor.matmul(psT[lo, :], lhsT=wth[lo, :], rhs=xb[lo, hh])
            nc.tensor.matmul(psT[hi, :], lhsT=wth[hi, :], rhs=xb[hi, hh])
            nc.vector.tensor_copy(out=T_sb[:, hh], in_=psT[:])

        s.update(T_sb=T_sb, PhiT_b=PhiT_b, PhiT_c=PhiT_c)
        s["P2"] = acts.tile([128, N], BF, tag="P2", name="P2")

    def gtstage(p):
        """GT for pair p — off the first-exp critical path."""
        xb = xball[:, p, :]
        s = st[p]
        psGT_b = psSm.tile([128, NH], F32, tag="psSm", name="psGT_b")
        psGT_c = psSm.tile([128, NH], F32, tag="psSm", name="psGT_c")
        for m in range(NK):
            mm = slice(m * 128, (m + 1) * 128)
            cc = slice(m * C, (m + 1) * C)
            nc.tensor.matmul(psGT_b[:, cc], lhsT=xb[lo, mm], rhs=wgv[lo, :])
            nc.tensor.matmul(psGT_c[:, cc], lhsT=xb[hi, mm], rhs=wgv[hi, :])
        GT_b = acts.tile([128, NH], BF, tag="GT_b", name="GT_b")
        GT_c = acts.tile([128, NH], BF, tag="GT_c", name="GT_c")
        nc.vector.tensor_copy(out=GT_b[:], in_=psGT_b[:])
        nc.vector.tensor_copy(out=GT_c[:], in_=psGT_c[:])
        s.update(GT_b=GT_b, GT_c=GT_c)

    def p2_quarter(p, j):
        """P2 column-quarter j (256 k's) for pair p, col-split by batch."""
        s = st[p]
        jj = slice(j * 256, (j + 1) * 256)
        psP2 = psO_pool.tile([128, 256], F32, tag="psO", name="psP2")
        for m in range(NK):
            cc = slice(m * C, (m + 1) * C)
            nc.tensor.matmul(psP2[lo, :], lhsT=acts_slice(s, "PhiT_b", cc),
                             rhs=wmk_q[j][:, m, :],
                             start=(m == 0), stop=(m == NK - 1))
            nc.tensor.matmul(psP2[hi, :], lhsT=acts_slice(s, "PhiT_c", cc),
                             rhs=wmk_q[j][:, m, :],
                             start=(m == 0), stop=(m == NK - 1))
        nc.vector.tensor_copy(out=s["P2"][:, jj], in_=psP2[:])

    def acts_slice(s, key, cc):
        return s[key][:, cc]

    def alloc_e(p):
        s = st[p]
        s["E_b"] = bigacts.tile([128, NK, N], BF, tag="E_b", name="E_b")
        s["E_c"] = bigacts.tile([128, NK, N], BF, tag="E_c", name="E_c")
        s["D_b"] = acts.tile([128, NK], F32, tag="D_b", name="D_b")
        s["D_c"] = acts.tile([128, NK], F32, tag="D_c", name="D_c")
        s["R_b"] = acts.tile([128, NK], F32, tag="R_b", name="R_b")
        s["R_c"] = acts.tile([128, NK], F32, tag="R_c", name="R_c")
        s["GTs_b"] = acts.tile([128, NH], BF, tag="GTs_b", name="GTs_b")
        s["GTs_c"] = acts.tile([128, NH], BF, tag="GTs_c", name="GTs_c")

    def s_exp_chunk(p, k):
        """S matmuls + exp (fused row-sum) for k-chunk of pair p.

        Each batch's [128, 512] matmul is col-split into two M=64 pieces so
        all four PE quadrants run concurrently (row-only-split matmul pairs
        do NOT overlap — col groups get their own XBUS streams, row groups
        share one).  Output layout in PSUM is unchanged: partition q of the
        chunk still holds k-index k*128+q.
        """
        s = st[p]
        klo = slice(k * 128, k * 128 + 64)
        khi = slice(k * 128 + 64, (k + 1) * 128)
        psS_b = psS.tile([128, N], F32, tag="psS", name="psS_b")
        psS_c = psS.tile([128, N], F32, tag="psS", name="psS_c")
        last_s_mm = [None]
        for h in range(2):
            hh = slice(h * NH, (h + 1) * NH)
            nc.tensor.matmul(psS_b[lo, hh], lhsT=s["P2"][lo, klo],
                             rhs=s["T_sb"][lo, hh])
            nc.tensor.matmul(psS_b[hi, hh], lhsT=s["P2"][lo, khi],
                             rhs=s["T_sb"][lo, hh])
            nc.tensor.matmul(psS_c[lo, hh], lhsT=s["P2"][hi, klo],
                             rhs=s["T_sb"][hi, hh])
            last_s_mm[0] = nc.tensor.matmul(
                psS_c[hi, hh], lhsT=s["P2"][hi, khi],
                rhs=s["T_sb"][hi, hh])
        nc.scalar.activation(s["E_b"][:, k, :], psS_b[:], EXP,
                             accum_out=s["D_b"][:, k:k + 1])
        nc.scalar.activation(s["E_c"][:, k, :], psS_c[:], EXP,
                             accum_out=s["D_c"][:, k:k + 1])
        return last_s_mm[0]

    def gts_chunkwise_init(p):
        """Allocate pair p's O accumulator banks (R/GTs live in alloc_e)."""
        s = st[p]
        s["psO"] = [psO_pool.tile([128, NH], F32, tag="psO", name=f"psO{h}")
                    for h in range(2)]

    def gts_chunk(p, k):
        s = st[p]
        cc = slice(k * C, (k + 1) * C)
        nc.vector.reciprocal(s["R_b"][:, k:k + 1], s["D_b"][:, k:k + 1])
        nc.vector.reciprocal(s["R_c"][:, k:k + 1], s["D_c"][:, k:k + 1])
        nc.vector.tensor_scalar_mul(s["GTs_b"][:, cc], s["GT_b"][:, cc],
                                    s["R_b"][:, k:k + 1])
        nc.vector.tensor_scalar_mul(s["GTs_c"][:, cc], s["GT_c"][:, cc],
                                    s["R_c"][:, k:k + 1])

    def o_chunk(p, m, after=None):
        """O accumulation m-chunk for pair p (both halves, col-split).
        `after`: instruction the first O matmul must follow in the PE
        stream (the scheduler's warm-timing model otherwise places cold O
        matmuls ahead of the next S chunk, stalling the exp chain)."""
        from concourse.bass import _add_dep_helper
        s = st[p]
        cc = slice(m * C, (m + 1) * C)
        for h in range(2):
            hh = slice(h * NH, (h + 1) * NH)
            mm1 = nc.tensor.matmul(s["psO"][h][lo, :], lhsT=s["GTs_b"][:, cc],
                                   rhs=s["E_b"][:, m, hh],
                                   start=(m == 0), stop=(m == NK - 1))
            if after is not None:
                _add_dep_helper(mm1.ins, after.ins,
                                reason="O chunk after next S chunk")
                after = None
            nc.tensor.matmul(s["psO"][h][hi, :], lhsT=s["GTs_c"][:, cc],
                             rhs=s["E_c"][:, m, hh],
                             start=(m == 0), stop=(m == NK - 1))

    def finish(p):
        """O copyback, mask, residual add, out DMA for pair p."""
        s = st[p]
        O_sb = acts.tile([128, N], BF, tag="O_sb", name="O_sb")
        for h in range(2):
            hh = slice(h * NH, (h + 1) * NH)
            nc.vector.tensor_copy(out=O_sb[:, hh], in_=s["psO"][h][:])
        out_sb = acts.tile([128, N], F32, tag="out_sb", name="out_sb")
        for h in range(2):
            hh = slice(h * NH, (h + 1) * NH)
            psM = psSm.tile([128, NH], F32, tag="psSm", name="psM")
            nc.tensor.matmul(psM[lo, :], lhsT=wma[lo, :], rhs=O_sb[lo, hh])
            nc.tensor.matmul(psM[hi, :], lhsT=wma[hi, :], rhs=O_sb[hi, hh])
            nc.vector.tensor_tensor(out_sb[:, hh], psM[:],
                                    xfall[:, p, hh], mybir.AluOpType.add)
        nc.gpsimd.dma_start(out_e[p * 128:(p + 1) * 128, :], out_sb[:])

    # ---- software pipeline over the pairs ----
    # Pair 0's O rides pair 1's exp phase; pair 1's O runs in the tail.
    # Next pair's stage-1/P2 fills the current phase at low priority.
    # NOTE: DVE is strict FIFO — any DVE op whose producer resolves late
    # head-of-line-blocks later critical copies, so GT stays at normal
    # priority right after stage-1.
    def low():
        return tc.high_priority(offset=-100000)

    stage1(0)
    gtstage(0)
    alloc_e(0)
    gts_chunkwise_init(0)
    for j in range(4):
        p2_quarter(0, j)
    for p in range(NPAIRS):
        nxt = p + 1
        for k in range(NK):
            s_mm = s_exp_chunk(p, k)
            gts_chunk(p, k)
            if p > 0:
                with low():
                    if k >= 1:
                        o_chunk(p - 1, k - 1, after=s_mm)
                    if k == NK - 1:
                        o_chunk(p - 1, NK - 1, after=None)
            if nxt < NPAIRS:
                if k == 1:
                    with low():
                        stage1(nxt)
                        gtstage(nxt)
                if k == 3:
                    alloc_e(nxt)
                if 2 <= k < 6:
                    with low():
                        p2_quarter(nxt, k - 2)
            if p > 0 and k == NK - 1:
                with low():
                    finish(p - 1)
        if nxt < NPAIRS:
            gts_chunkwise_init(nxt)
    for m in range(NK):
        o_chunk(NPAIRS - 1, m)
    finish(NPAIRS - 1)


def _eliminate_redundant_waits(nc):
    """Transitive redundant-wait elimination over the final BIR stream.

    Tile's sem assignment is per-proc minimal but NOT transitively minimal:
    e.g. a matmul reusing a PSUM slot gets both (ACT >= k) [reader done] and
    (PE >= p) [previous writer done] waits, although observing ACT >= k
    already implies PE >= p (the reader waited on the writer).  The extra
    same-engine waits serialize the PE pipeline (no back-to-back streaming,
    no quadrant concurrency).

    Soundness relies on per-queue in-order completion (PE pc-monotone,
    ACT/DVE strict FIFO):  observing sem s >= v implies the v-th
    incrementing instruction and its whole same-queue prefix completed,
    hence all THEIR increments fired and all their waits were satisfied.
    """
    blocks = list(nc.m.functions[0].blocks)
    seq = []
    for blk in blocks:
        for ins in blk.instructions:
            seq.append(ins)

    def queue_key(ins):
        si = getattr(ins, "sync_info", None)
        nm = type(ins).__name__
        if nm in ("InstDMACopy", "InstTensorLoad", "InstTensorSave"):
            if si and si.on_update:
                return "Q" + si.on_update[0].ant_name
        return "E" + str(ins.engine)

    sem_count = {}
    incpoints = {}
    qpos = {}
    qidx = {}
    for ins in seq:
        qk = queue_key(ins)
        i = qpos.get(qk, 0)
        qidx[id(ins)] = (qk, i)
        qpos[qk] = i + 1
        si = getattr(ins, "sync_info", None)
        if si and si.on_update:
            for u in si.on_update:
                s = u.ant_name
                v = sem_count.get(s, 0) + (u.update_value or 1)
                sem_count[s] = v
                incpoints.setdefault(s, []).append((v, qk, i))

    per_queue = {}
    for ins in seq:
        qk, i = qidx[id(ins)]
        per_queue.setdefault(qk, []).append(ins)

    def merge(a, b):
        if not b:
            return a
        out = dict(a)
        for k, v in b.items():
            if out.get(k, 0) < v:
                out[k] = v
        return out

    comp_cache = {}

    def know_comp(qk, i):
        if i < 0:
            return {}
        key = (qk, i)
        if key in comp_cache:
            return comp_cache[key]
        know = dict(know_comp(qk, i - 1))
        ins = per_queue[qk][i]
        si = getattr(ins, "sync_info", None)
        if si:
            for w in (si.on_wait or []):
                if know.get(w.ant_name, 0) < w.wait_value:
                    know[w.ant_name] = w.wait_value
                    know = merge(know, know_from_obs(w.ant_name, w.wait_value))
        comp_cache[key] = know
        return know

    obs_cache = {}

    def _dma_sem(sem):
        return "DMA" in sem

    def know_from_obs(sem, v):
        if _dma_sem(sem):
            return {}
        key = (sem, v)
        if key in obs_cache:
            return obs_cache[key]
        obs_cache[key] = {}
        pts = incpoints.get(sem, [])
        know = {}
        if pts and all(q == pts[0][1] for _, q, _ in pts):
            for cnt, qk, i in pts:
                if cnt >= v:
                    if qk.startswith("E"):
                        know = dict(know_comp(qk, i))
                    know[sem] = cnt
                    break
        obs_cache[key] = know
        return know

    import os
    mode = os.environ.get("KERNEL_ELIM", "self")
    self_only = (mode == "self")

    def _same_queue_sem(sem, qk):
        pts = incpoints.get(sem, [])
        return bool(pts) and all(q == qk for _, q, _ in pts)

    dropped = 0
    kept = 0
    for qk, insts in per_queue.items():
        if not qk.startswith("E"):
            continue
        know = {}
        for ins in insts:
            si = getattr(ins, "sync_info", None)
            if not si:
                continue
            if type(ins).__name__ in ("InstDMACopy", "InstTensorLoad",
                                      "InstTensorSave", "InstTriggeredCopy"):
                continue
            waits = list(si.on_wait or [])
            if waits:
                changed = True
                waitset = waits[:]
                while changed:
                    changed = False
                    for w in waitset[:]:
                        if self_only and not _same_queue_sem(w.ant_name, qk):
                            continue
                        base = dict(know)
                        for w2 in waitset:
                            if w2 is w:
                                continue
                            base[w2.ant_name] = max(
                                base.get(w2.ant_name, 0), w2.wait_value)
                            base = merge(
                                base, know_from_obs(w2.ant_name, w2.wait_value))
                        if base.get(w.ant_name, 0) >= w.wait_value:
                            waitset.remove(w)
                            dropped += 1
                            changed = True
                            break
                for w in waitset:
                    kept += 1
                    know[w.ant_name] = max(know.get(w.ant_name, 0), w.wait_value)
                    know = merge(know, know_from_obs(w.ant_name, w.wait_value))
                if len(waitset) != len(waits):
                    ins.sync_info = mybir.SyncInfo(
                        on_wait=waitset, on_update=list(si.on_update or []))
    return dropped, kept


_SPLIT_WAIT_TYPES = {
    "InstMatmult", "InstTensorTensor", "InstTensorCopy", "InstActivation",
    "InstTensorScalarPtr", "InstTensorScalar", "InstReciprocal",
    "InstTensorReduce", "InstMemSet", "InstLdweights", "InstTranspose",
    "InstTensorTensorScan", "InstSelect", "InstCopy", "InstDMACopy",
    "InstTensorLoad", "InstTensorSave", "InstDrain",
}


def _split_matmul_waits(nc):
    """Walrus's TRN2 codegen allows at most one sync-wait per compute
    instruction.  Hoist every wait of a multi-wait instruction onto NoOps
    placed right before it on the same engine — the NX sequencer executes
    them in order, so semantics are identical.
    """
    cnt = 0
    for blk in nc.m.functions[0].blocks:
        insts = blk.instructions
        new = []
        for ins in insts:
            si = getattr(ins, "sync_info", None)
            if (type(ins).__name__ in _SPLIT_WAIT_TYPES and si is not None
                    and si.on_wait and len(si.on_wait) > 1):
                for j, w in enumerate(si.on_wait):
                    nop = mybir.InstNoOp(
                        name=f"{ins.name}-w{j}",
                        engine=ins.engine,
                        sync_info=mybir.SyncInfo(on_wait=[w], on_update=[]),
                        bass_nofuse=True,
                    )
                    new.append(nop)
                ins.sync_info = mybir.SyncInfo(
                    on_wait=[], on_update=list(si.on_update))
                cnt += 1
            new.append(ins)
        blk.instructions = new
    return cnt



def build_nc_full():
    nc = bass.Bass()
    # Per-core inputs.  x rows: pair p occupies partitions [0:128) as
    # (batch 2p on 0-63, batch 2p+1 on 64-127) after slicing [p*128:(p+1)*128).
    x32 = nc.declare_dram_parameter("x32", [BPC * C, N], F32, isOutput=False)
    x16 = nc.declare_dram_parameter("x16", [BPC * C, N], BF, isOutput=False)
    # four [64,64] conv weights, transposed, partition-duplicated, packed
    # along the free axis: [wth | wph | wgv | wma]
    wsmallT = nc.declare_dram_parameter("wsmallT", [128, 4 * C], BF,
                                        isOutput=False)
    # w_mk^T in k-quarter-major layout [4*N, 256]
    wmkhT = nc.declare_dram_parameter("wmkhT", [4 * N, 256], BF,
                                      isOutput=False)
    out_e = nc.declare_dram_parameter("out", [BPC * C, N], F32, isOutput=True)

    with tile.TileContext(nc) as tc:
        with (
            tc.tile_pool(name="consts", bufs=1) as consts,
            tc.tile_pool(name="acts", bufs=2) as acts,
            tc.tile_pool(name="bigacts", bufs=2) as bigacts,
            tc.tile_pool(name="psO", bufs=2, space="PSUM") as psO_pool,
            tc.tile_pool(name="psS", bufs=2, space="PSUM") as psS,
            tc.tile_pool(name="psSm", bufs=2, space="PSUM") as psSm,
        ):
            _build_body(nc, tc, consts, acts, bigacts, psO_pool, psS, psSm,
                        x32, x16, wsmallT, wmkhT, out_e)
    import os
    if os.environ.get("KERNEL_ELIM", "1") != "0":
        d, k = _eliminate_redundant_waits(nc)
        print(f"wait elimination: dropped {d}, kept {k}")
    _split_matmul_waits(nc)
    return nc


def _prep_weights(w_phi, w_theta, w_g, w_mask, w_mv, w_mk):
    bf = ml_dtypes.bfloat16

    def dup(a):  # [64, 64] -> [128, 64], duplicated on both partition halves
        return np.ascontiguousarray(np.concatenate([a, a], axis=0)).astype(bf)

    w_gv = (w_mv.astype(np.float64) @ w_g.astype(np.float64)).astype(np.float32)
    wsmall = np.concatenate(
        [dup(w_theta.T), dup(w_phi.T), dup(w_gv.T), dup(w_mask.T)], axis=1)
    # w_mk^T [m, k] -> k-quarter-major [4, m, 256] -> [4*m, 256]
    wmkT = np.ascontiguousarray(w_mk.T).astype(bf)
    wmkh = np.ascontiguousarray(
        wmkT.reshape(N, 4, 256).transpose(1, 0, 2)).reshape(4 * N, 256)
    return {
        "wsmallT": np.ascontiguousarray(wsmall),
        "wmkhT": wmkh,
    }


def kernel(x, w_phi, w_theta, w_g, w_mask, w_mv, w_mk, _trace=False):
    bf = ml_dtypes.bfloat16
    x = np.asarray(x, dtype=np.float32)
    weights = _prep_weights(np.asarray(w_phi, np.float32),
                            np.asarray(w_theta, np.float32),
                            np.asarray(w_g, np.float32),
                            np.asarray(w_mask, np.float32),
                            np.asarray(w_mv, np.float32),
                            np.asarray(w_mk, np.float32))

    xr = x.reshape(B, C, N)
    in_maps = []
    for i in range(NCORES):
        shard = np.ascontiguousarray(xr[i * BPC:(i + 1) * BPC]).reshape(BPC * C, N)
        m = {"x32": shard, "x16": shard.astype(bf)}
        m.update(weights)
        in_maps.append(m)

    nc = build_nc_full()
    res = run_bass_kernel_spmd(nc, in_maps, list(range(NCORES)), trace=_trace)
    outs = [np.asarray(res.results[i]["out"]).reshape(BPC, C, HH, WW)
            for i in range(NCORES)]
    full = np.concatenate(outs, axis=0)
    if _trace:
        return full, res
    return full



# revision 4
# speedup vs baseline: 1.1141x; 1.1141x over previous
"""Trainium2 Bass kernel for the non-local attention block (dense_transformer).

Reference computation per batch item b (x: [B=32, C=64, H=32, W=32], N=1024):
    xf    = x[b] reshaped [C, N]
    phi   = w_phi   @ xf                     [C, N]
    theta = (w_theta @ xf)^T                 [N, C]
    g     = (w_g @ xf)^T @ w_mv^T            [N, C]
    att   = theta @ phi                      [N, N]
    att   = att @ w_mk^T                     [N, N]
    att   = softmax(att, axis over rows n)
    out   = att @ g                          [N, C]
    final = w_mask @ out^T + xf              [C, N]

Key algebraic restructure: (theta @ phi) @ w_mk^T == theta @ (phi @ w_mk^T),
which removes the N^3 matmul.  The softmax denominator divide is folded into
the small g factor (64 wide).

Per-core layout (data-parallel, 4 batch items per core, as 2 stacked pairs
occupying the 128 SBUF partitions: batch "b" on partitions 0-63, batch "c"
on 64-127).  The four [64,64] conv weights are replicated into [128,128]
BLOCK-DIAGONAL matrices so one full-array matmul computes both batches at
once (stage-1 PhiT/T/GT and the final mask):
    PhiT_il = x_il^T @ bd(w_phi^T)    [n, c2]   8 MMs/pair (m-chunks)
    T       = bd(w_theta^T)^T @ x_il  [c2, n]   2 MMs/pair
    GT_il   = x_il^T @ bd(w_gv^T)     [n, c2]   8 MMs/pair
    P2      = PhiT_il^T @ w_mk^T      [c2, k]   32 full-array MMs/pair
    S       = P2^T @ T (quadrant 4x)  [k, n]    = att2^T, 8 MMs/chunk
    E       = exp(S)  (ScalarE, fused row-sum accum -> D)
    GTs     = GT_il * (1/D)           (fold softmax divide into g)
    O       = GTs^T @ E               [c2, k]   col-split, accum m-chunks
    final   = bd(w_mask^T)^T @ O + x  (mask matmul IN-PLACE into the psO
                                       banks, so no extra PSUM pool)

Schedule (vs the 91-101us baseline, measured bottlenecks from NTFF trace):
  - Input DMA issues are spread across the Sync/GpSimd/Vector engine rings
    (each dma_start costs ~1.2us of sequencer issue time; serialized on one
    ring they delayed all transfers to t=9.3us and the last input to 22us).
    x16 and wmk-quarter-0 gate the pipeline start and go first on their
    rings; xfall (residual, needed late) goes last.
  - Only P2 quarter 0 runs in the lead-in; quarters 1-3 are deferred into
    the exp phase (low priority + dep edges after the S chain).
  - Each pair's O accumulation rides its OWN exp phase (o_chunk(p, k-1)
    after chunk k's S matmuls), so the tail is just the last O chunk +
    mask + out-DMA instead of a 16us cold-PE epilogue.
  - finish(p) runs at normal priority at the next pair's k==0 so the psO
    banks hand off in time and the out-DMA leaves mid-kernel.
  - S chunk issues batch-b's 4 quadrant MMs before batch-c's so exp_b's
    operands complete ~430ns earlier (ACT is the bottleneck engine; its
    cadence is set by when the S matmuls land).

All matmul operands bf16 (PE full rate); PSUM fp32; softmax sum fp32 via
activation accum_out.  PSUM budget (8 banks): psS 2 slots x [128,1024] = 4;
psO (O accum + in-place mask) 2 slots x [128,512] = 2; psSm (stage-1 psums
+ P2 quarters) 2 slots x [128,512] = 2.

Post-passes: _eliminate_redundant_waits strips Tile's transitively-implied
same-engine sem waits; _split_matmul_waits hoists remaining multi-wait
instructions onto single-wait NoOps (TRN2 walrus allows one sync-wait per
instruction).

Rel err ~1.0e-2 vs the fp32 reference (bf16 matmul rounding).
"""

import numpy as np
import ml_dtypes

import concourse.bass as bass
import concourse.mybir as mybir
import concourse.tile as tile
from concourse.bass_utils import run_bass_kernel_spmd

BF = mybir.dt.bfloat16
F32 = mybir.dt.float32
EXP = mybir.ActivationFunctionType.Exp

B, C, HH, WW = 32, 64, 32, 32
N = HH * WW          # 1024
NCORES = 8
BPC = B // NCORES    # 4 batch items per core
NPAIRS = BPC // 2    # 2 stacked pairs per core
NK = N // 128        # 8 chunks of 128 along the N dimension
NH = 512             # matmul free-dim half (one PSUM bank)


def _build_body(nc, tc, consts, acts, bigacts, psO_pool, psS, psSm,
                xall32, xall16, wbdT, wmkhT, out_e):
    from concourse.bass import _add_dep_helper
    lo = slice(0, 64)
    hi = slice(64, 128)

    # ---- PE warmup: dummy matmuls on a zeroed tile keep the PE busy for
    # the HAM SHORT window (~3.4us) while the input DMAs run, so real work
    # starts at 2.4 GHz instead of 1.2.
    warm_in = consts.tile([128, 256], BF, tag="warm_in")
    nc.vector.memset(warm_in[:], 0.0)
    warm_ps = psS.tile([128, N], F32, tag="psS", name="warm_ps")
    for i in range(40):
        nc.tensor.matmul(warm_ps[:, 0:128], lhsT=warm_in[:, 0:128],
                         rhs=warm_in[:, 128:256])

    # ---- input DMAs, spread across engine rings so the ~1.2us-per-issue
    # sequencer cost parallelizes.  Per-ring order = HW completion order.
    wbd = consts.tile([128, 4 * 128], BF, tag="wbd")
    nc.sync.dma_start(wbd[:], wbdT[:])
    xball = consts.tile([128, NPAIRS, N], BF, tag="xball")
    nc.sync.dma_start(xball[:], xall16.rearrange("(p q) n -> q p n", p=NPAIRS))
    wth = wbd[:, 0:128]
    wph = wbd[:, 128:256]
    wgv = wbd[:, 256:384]
    wma = wbd[:, 384:512]

    # w_mk^T in k-quarter-major DRAM layout: one piece per quarter so P2
    # quarter j starts as soon as piece j lands.
    wmk_q = []
    for j in range(4):
        t = consts.tile([128, NK, 256], BF, tag=f"wmkq{j}")
        wmk_q.append(t)
    nc.gpsimd.dma_start(
        wmk_q[0][:], wmkhT[0:N, :].rearrange("(mc q) k -> q mc k", mc=NK))
    nc.scalar.dma_start(
        wmk_q[1][:], wmkhT[N:2 * N, :].rearrange("(mc q) k -> q mc k", mc=NK))
    nc.gpsimd.dma_start(
        wmk_q[2][:], wmkhT[2 * N:3 * N, :].rearrange("(mc q) k -> q mc k", mc=NK))
    nc.scalar.dma_start(
        wmk_q[3][:], wmkhT[3 * N:4 * N, :].rearrange("(mc q) k -> q mc k", mc=NK))
    xfall = consts.tile([128, NPAIRS, N], F32, tag="xfall")
    nc.gpsimd.dma_start(xfall[:], xall32.rearrange("(p q) n -> q p n", p=NPAIRS))

    st = [dict() for _ in range(NPAIRS)]

    def stage1(p):
        """PhiT_il + T for pair p (block-diagonal weights, full-array MMs)."""
        xb = xball[:, p, :]
        s = st[p]
        PhiT = acts.tile([128, NK, 128], BF, tag="PhiT", name="PhiT")
        for g in range(2):
            psPh = psSm.tile([128, 4, 128], F32, tag="psSm", name="psPh")
            for mq in range(4):
                m = g * 4 + mq
                nc.tensor.matmul(psPh[:, mq, :],
                                 lhsT=xb[:, m * 128:(m + 1) * 128], rhs=wph[:])
            nc.vector.tensor_copy(out=PhiT[:, g * 4:(g + 1) * 4, :], in_=psPh[:])
        T_sb = acts.tile([128, N], BF, tag="T_sb", name="T_sb")
        for h in range(2):
            hh = slice(h * NH, (h + 1) * NH)
            psT = psSm.tile([128, NH], F32, tag="psSm", name="psT")
            nc.tensor.matmul(psT[:], lhsT=wth[:], rhs=xb[:, hh])
            nc.vector.tensor_copy(out=T_sb[:, hh], in_=psT[:])
        s.update(PhiT=PhiT, T_sb=T_sb)
        s["P2"] = acts.tile([128, N], BF, tag="P2", name="P2")

    def gtstage(p):
        """GT_il for pair p — off the first-exp critical path."""
        xb = xball[:, p, :]
        s = st[p]
        GT = acts.tile([128, NK, 128], BF, tag="GT", name="GT")
        for g in range(2):
            psG = psSm.tile([128, 4, 128], F32, tag="psSm", name="psG")
            for mq in range(4):
                m = g * 4 + mq
                nc.tensor.matmul(psG[:, mq, :],
                                 lhsT=xb[:, m * 128:(m + 1) * 128], rhs=wgv[:])
            nc.vector.tensor_copy(out=GT[:, g * 4:(g + 1) * 4, :], in_=psG[:])
        s["GT"] = GT

    def p2_quarter(p, j, after=None):
        """P2 column-quarter j (256 k's) for pair p — one full-array MM per
        m-chunk (both batches via the interleaved PhiT layout)."""
        s = st[p]
        psP2 = psSm.tile([128, 256], F32, tag="psSm", name="psP2")
        for m in range(NK):
            mm = nc.tensor.matmul(psP2[:], lhsT=s["PhiT"][:, m, :],
                                  rhs=wmk_q[j][:, m, :],
                                  start=(m == 0), stop=(m == NK - 1))
            if after is not None:
                _add_dep_helper(mm.ins, after.ins, reason="P2 after S chain")
                after = None
        nc.vector.tensor_copy(out=s["P2"][:, j * 256:(j + 1) * 256], in_=psP2[:])

    def alloc_e(p):
        s = st[p]
        s["E_b"] = bigacts.tile([128, NK, N], BF, tag="E_b", name="E_b")
        s["E_c"] = bigacts.tile([128, NK, N], BF, tag="E_c", name="E_c")
        s["D"] = acts.tile([128, NK, 2], F32, tag="D", name="D")
        s["R"] = acts.tile([128, NK, 2], F32, tag="R", name="R")
        s["GTs"] = acts.tile([128, NK, 128], BF, tag="GTs", name="GTs")

    def alloc_o(p):
        s = st[p]
        s["psO"] = [psO_pool.tile([128, NH], F32, tag="psO", name=f"psO{h}")
                    for h in range(2)]

    def s_exp_chunk(p, k):
        """S matmuls + exp (fused row-sum) for k-chunk of pair p.

        Four 64x64 PE quadrants via tile-position packing; batch b's four
        matmuls issue first so exp_b's operands land ~430ns earlier.
        """
        s = st[p]
        klo = slice(k * 128, k * 128 + 64)
        khi = slice(k * 128 + 64, (k + 1) * 128)
        psS_b = psS.tile([128, N], F32, tag="psS", name="psS_b")
        psS_c = psS.tile([128, N], F32, tag="psS", name="psS_c")
        for h in range(2):
            hh = slice(h * NH, (h + 1) * NH)
            nc.tensor.matmul(psS_b[lo, hh], lhsT=s["P2"][lo, klo],
                             rhs=s["T_sb"][lo, hh])
            nc.tensor.matmul(psS_b[hi, hh], lhsT=s["P2"][lo, khi],
                             rhs=s["T_sb"][lo, hh])
        last = None
        for h in range(2):
            hh = slice(h * NH, (h + 1) * NH)
            nc.tensor.matmul(psS_c[lo, hh], lhsT=s["P2"][hi, klo],
                             rhs=s["T_sb"][hi, hh])
            last = nc.tensor.matmul(psS_c[hi, hh], lhsT=s["P2"][hi, khi],
                                    rhs=s["T_sb"][hi, hh])
        nc.scalar.activation(s["E_b"][:, k, :], psS_b[:], EXP,
                             accum_out=s["D"][:, k, 0:1])
        nc.scalar.activation(s["E_c"][:, k, :], psS_c[:], EXP,
                             accum_out=s["D"][:, k, 1:2])
        return last

    def gts_chunk(p, k):
        s = st[p]
        nc.vector.reciprocal(s["R"][:, k, :], s["D"][:, k, :])
        nc.vector.tensor_scalar_mul(s["GTs"][:, k, 0:64], s["GT"][:, k, 0:64],
                                    s["R"][:, k, 0:1])
        nc.vector.tensor_scalar_mul(s["GTs"][:, k, 64:128],
                                    s["GT"][:, k, 64:128], s["R"][:, k, 1:2])

    def o_chunk(p, m, after=None):
        """O accumulation m-chunk for pair p (col-split, both batches).
        `after`: S matmul the first O matmul must follow in the PE stream
        (keeps cold O matmuls from head-of-line-blocking the exp chain)."""
        s = st[p]
        for h in range(2):
            hh = slice(h * NH, (h + 1) * NH)
            mm = nc.tensor.matmul(s["psO"][h][lo, :],
                                  lhsT=s["GTs"][:, m, 0:64],
                                  rhs=s["E_b"][:, m, hh],
                                  start=(m == 0), stop=(m == NK - 1))
            if after is not None:
                _add_dep_helper(mm.ins, after.ins, reason="O chunk after S")
                after = None
            nc.tensor.matmul(s["psO"][h][hi, :], lhsT=s["GTs"][:, m, 64:128],
                             rhs=s["E_c"][:, m, hh],
                             start=(m == 0), stop=(m == NK - 1))

    def finish(p, after=None):
        """O copyback, in-place mask matmul (reuses psO banks), residual
        add, out DMA for pair p."""
        s = st[p]
        O_sb = acts.tile([128, N], BF, tag="O_sb", name="O_sb")
        for h in range(2):
            hh = slice(h * NH, (h + 1) * NH)
            nc.vector.tensor_copy(out=O_sb[:, hh], in_=s["psO"][h][:])
        out_sb = acts.tile([128, N], F32, tag="out_sb", name="out_sb")
        for h in range(2):
            hh = slice(h * NH, (h + 1) * NH)
            mm = nc.tensor.matmul(s["psO"][h][:], lhsT=wma[:], rhs=O_sb[:, hh],
                                  start=True, stop=True)
            if after is not None:
                _add_dep_helper(mm.ins, after.ins, reason="mask after S")
                after = None
            nc.vector.tensor_tensor(out_sb[:, hh], s["psO"][h][:],
                                    xfall[:, p, hh], mybir.AluOpType.add)
        nc.gpsimd.dma_start(out_e[p * 128:(p + 1) * 128, :], out_sb[:])

    def low():
        return tc.high_priority(offset=-100000)

    # ---- software pipeline ----
    stage1(0)
    alloc_e(0)
    p2_quarter(0, 0)
    with low():
        gtstage(0)
    for p in range(NPAIRS):
        nxt = p + 1
        alloc_o(p)
        for k in range(NK):
            s_mm = s_exp_chunk(p, k)
            if k == 0 and p > 0:
                with low():
                    o_chunk(p - 1, NK - 1, after=s_mm)
                finish(p - 1, after=s_mm)
            gts_chunk(p, k)
            if k >= 1:
                with low():
                    o_chunk(p, k - 1, after=s_mm)
            if k <= 2:
                with low():
                    p2_quarter(p, k + 1, after=s_mm)
            if nxt < NPAIRS:
                if k == 3:
                    with low():
                        stage1(nxt)
                if k == 4:
                    alloc_e(nxt)
                    with low():
                        gtstage(nxt)
                if k == 5:
                    with low():
                        p2_quarter(nxt, 0, after=s_mm)
    o_chunk(NPAIRS - 1, NK - 1)
    finish(NPAIRS - 1)


def _eliminate_redundant_waits(nc):
    """Transitive redundant-wait elimination over the final BIR stream.

    Tile's sem assignment is per-proc minimal but NOT transitively minimal:
    e.g. a matmul reusing a PSUM slot gets both (ACT >= k) [reader done] and
    (PE >= p) [previous writer done] waits, although observing ACT >= k
    already implies PE >= p (the reader waited on the writer).  The extra
    same-engine waits serialize the PE pipeline (no back-to-back streaming,
    no quadrant concurrency).

    Soundness relies on per-queue in-order completion (PE pc-monotone,
    ACT/DVE strict FIFO):  observing sem s >= v implies the v-th
    incrementing instruction and its whole same-queue prefix completed,
    hence all THEIR increments fired and all their waits were satisfied.
    """
    blocks = list(nc.m.functions[0].blocks)
    seq = []
    for blk in blocks:
        for ins in blk.instructions:
            seq.append(ins)

    def queue_key(ins):
        si = getattr(ins, "sync_info", None)
        nm = type(ins).__name__
        if nm in ("InstDMACopy", "InstTensorLoad", "InstTensorSave"):
            if si and si.on_update:
                return "Q" + si.on_update[0].ant_name
        return "E" + str(ins.engine)

    sem_count = {}
    incpoints = {}
    qpos = {}
    qidx = {}
    for ins in seq:
        qk = queue_key(ins)
        i = qpos.get(qk, 0)
        qidx[id(ins)] = (qk, i)
        qpos[qk] = i + 1
        si = getattr(ins, "sync_info", None)
        if si and si.on_update:
            for u in si.on_update:
                s = u.ant_name
                v = sem_count.get(s, 0) + (u.update_value or 1)
                sem_count[s] = v
                incpoints.setdefault(s, []).append((v, qk, i))

    per_queue = {}
    for ins in seq:
        qk, i = qidx[id(ins)]
        per_queue.setdefault(qk, []).append(ins)

    def merge(a, b):
        if not b:
            return a
        out = dict(a)
        for k, v in b.items():
            if out.get(k, 0) < v:
                out[k] = v
        return out

    comp_cache = {}

    def know_comp(qk, i):
        if i < 0:
            return {}
        key = (qk, i)
        if key in comp_cache:
            return comp_cache[key]
        know = dict(know_comp(qk, i - 1))
        ins = per_queue[qk][i]
        si = getattr(ins, "sync_info", None)
        if si:
            for w in (si.on_wait or []):
                if know.get(w.ant_name, 0) < w.wait_value:
                    know[w.ant_name] = w.wait_value
                    know = merge(know, know_from_obs(w.ant_name, w.wait_value))
        comp_cache[key] = know
        return know

    obs_cache = {}

    def _dma_sem(sem):
        return "DMA" in sem

    def know_from_obs(sem, v):
        if _dma_sem(sem):
            return {}
        key = (sem, v)
        if key in obs_cache:
            return obs_cache[key]
        obs_cache[key] = {}
        pts = incpoints.get(sem, [])
        know = {}
        if pts and all(q == pts[0][1] for _, q, _ in pts):
            for cnt, qk, i in pts:
                if cnt >= v:
                    if qk.startswith("E"):
                        know = dict(know_comp(qk, i))
                    know[sem] = cnt
                    break
        obs_cache[key] = know
        return know

    import os
    mode = os.environ.get("KERNEL_ELIM", "self")
    self_only = (mode == "self")

    def _same_queue_sem(sem, qk):
        pts = incpoints.get(sem, [])
        return bool(pts) and all(q == qk for _, q, _ in pts)

    dropped = 0
    kept = 0
    for qk, insts in per_queue.items():
        if not qk.startswith("E"):
            continue
        know = {}
        for ins in insts:
            si = getattr(ins, "sync_info", None)
            if not si:
                continue
            if type(ins).__name__ in ("InstDMACopy", "InstTensorLoad",
                                      "InstTensorSave", "InstTriggeredCopy"):
                continue
            waits = list(si.on_wait or [])
            if waits:
                changed = True
                waitset = waits[:]
                while changed:
                    changed = False
                    for w in waitset[:]:
                        if self_only and not _same_queue_sem(w.ant_name, qk):
                            continue
                        base = dict(know)
                        for w2 in waitset:
                            if w2 is w:
                                continue
                            base[w2.ant_name] = max(
                                base.get(w2.ant_name, 0), w2.wait_value)
                            base = merge(
                                base, know_from_obs(w2.ant_name, w2.wait_value))
                        if base.get(w.ant_name, 0) >= w.wait_value:
                            waitset.remove(w)
                            dropped += 1
                            changed = True
                            break
                for w in waitset:
                    kept += 1
                    know[w.ant_name] = max(know.get(w.ant_name, 0), w.wait_value)
                    know = merge(know, know_from_obs(w.ant_name, w.wait_value))
                if len(waitset) != len(waits):
                    ins.sync_info = mybir.SyncInfo(
                        on_wait=waitset, on_update=list(si.on_update or []))
    return dropped, kept


_SPLIT_WAIT_TYPES = {
    "InstMatmult", "InstTensorTensor", "InstTensorCopy", "InstActivation",
    "InstTensorScalarPtr", "InstTensorScalar", "InstReciprocal",
    "InstTensorReduce", "InstMemSet", "InstLdweights", "InstTranspose",
    "InstTensorTensorScan", "InstSelect", "InstCopy", "InstDMACopy",
    "InstTensorLoad", "InstTensorSave", "InstDrain",
}


def _split_matmul_waits(nc):
    """Walrus's TRN2 codegen allows at most one sync-wait per compute
    instruction.  Hoist every wait of a multi-wait instruction onto NoOps
    placed right before it on the same engine — the NX sequencer executes
    them in order, so semantics are identical.
    """
    cnt = 0
    for blk in nc.m.functions[0].blocks:
        insts = blk.instructions
        new = []
        for ins in insts:
            si = getattr(ins, "sync_info", None)
            if (type(ins).__name__ in _SPLIT_WAIT_TYPES and si is not None
                    and si.on_wait and len(si.on_wait) > 1):
                for j, w in enumerate(si.on_wait):
                    nop = mybir.InstNoOp(
                        name=f"{ins.name}-w{j}",
                        engine=ins.engine,
                        sync_info=mybir.SyncInfo(on_wait=[w], on_update=[]),
                        bass_nofuse=True,
                    )
                    new.append(nop)
                ins.sync_info = mybir.SyncInfo(
                    on_wait=[], on_update=list(si.on_update))
                cnt += 1
            new.append(ins)
        blk.instructions = new
    return cnt


def build_nc_full():
    nc = bass.Bass()
    # Per-core inputs.  x rows: pair p occupies partitions [0:128) as
    # (batch 2p on 0-63, batch 2p+1 on 64-127) after slicing.
    x32 = nc.declare_dram_parameter("x32", [BPC * C, N], F32, isOutput=False)
    x16 = nc.declare_dram_parameter("x16", [BPC * C, N], BF, isOutput=False)
    # four [128,128] block-diagonal conv weights packed along the free axis:
    # [bd(w_theta^T) | bd(w_phi^T) | bd(w_gv^T) | bd(w_mask^T)]
    wbdT = nc.declare_dram_parameter("wbdT", [128, 4 * 128], BF,
                                     isOutput=False)
    # w_mk^T in k-quarter-major layout [4*N, 256]
    wmkhT = nc.declare_dram_parameter("wmkhT", [4 * N, 256], BF,
                                      isOutput=False)
    out_e = nc.declare_dram_parameter("out", [BPC * C, N], F32, isOutput=True)

    with tile.TileContext(nc) as tc:
        with (
            tc.tile_pool(name="consts", bufs=1) as consts,
            tc.tile_pool(name="acts", bufs=2) as acts,
            tc.tile_pool(name="bigacts", bufs=2) as bigacts,
            tc.tile_pool(name="psO", bufs=2, space="PSUM") as psO_pool,
            tc.tile_pool(name="psS", bufs=2, space="PSUM") as psS,
            tc.tile_pool(name="psSm", bufs=2, space="PSUM") as psSm,
        ):
            _build_body(nc, tc, consts, acts, bigacts, psO_pool, psS, psSm,
                        x32, x16, wbdT, wmkhT, out_e)
    import os
    if os.environ.get("KERNEL_ELIM", "1") != "0":
        d, k = _eliminate_redundant_waits(nc)
        print(f"wait elimination: dropped {d}, kept {k}")
    _split_matmul_waits(nc)
    return nc


def _prep_weights(w_phi, w_theta, w_g, w_mask, w_mv, w_mk):
    bf = ml_dtypes.bfloat16
    z = np.zeros((C, C), np.float32)

    def bd(a):  # [64, 64] -> [128, 128] block-diagonal of a.T
        at = np.ascontiguousarray(a.T).astype(np.float32)
        return np.block([[at, z], [z, at]])

    w_gv = (w_mv.astype(np.float64) @ w_g.astype(np.float64)).astype(np.float32)
    wbd = np.concatenate(
        [bd(w_theta), bd(w_phi), bd(w_gv), bd(w_mask)], axis=1).astype(bf)
    # w_mk^T [m, k] -> k-quarter-major [4, m, 256] -> [4*m, 256]
    wmkT = np.ascontiguousarray(w_mk.T).astype(bf)
    wmkh = np.ascontiguousarray(
        wmkT.reshape(N, 4, 256).transpose(1, 0, 2)).reshape(4 * N, 256)
    return {
        "wbdT": np.ascontiguousarray(wbd),
        "wmkhT": wmkh,
    }


def kernel(x, w_phi, w_theta, w_g, w_mask, w_mv, w_mk, _trace=False):
    bf = ml_dtypes.bfloat16
    x = np.asarray(x, dtype=np.float32)
    weights = _prep_weights(np.asarray(w_phi, np.float32),
                            np.asarray(w_theta, np.float32),
                            np.asarray(w_g, np.float32),
                            np.asarray(w_mask, np.float32),
                            np.asarray(w_mv, np.float32),
                            np.asarray(w_mk, np.float32))

    xr = x.reshape(B, C, N)
    in_maps = []
    for i in range(NCORES):
        shard = np.ascontiguousarray(xr[i * BPC:(i + 1) * BPC]).reshape(BPC * C, N)
        m = {"x32": shard, "x16": shard.astype(bf)}
        m.update(weights)
        in_maps.append(m)

    nc = build_nc_full()
    res = run_bass_kernel_spmd(nc, in_maps, list(range(NCORES)), trace=_trace)
    outs = [np.asarray(res.results[i]["out"]).reshape(BPC, C, HH, WW)
            for i in range(NCORES)]
    full = np.concatenate(outs, axis=0)
    if _trace:
        return full, res
    return full
